# revision 1
# baseline (speedup 1.0000x reference)
"""Trainium2 Bass kernel: embedding gather + Conv1d(k=5,pad=2) + positional add.

Sequence-parallel across 8 NeuronCores; core c computes output tokens
[c*1024, (c+1)*1024) for all 8 batch rows. b_lin/b_conv/pos_table fold into
one per-position bias table on the host.

Per core, per batch row:
  1. 9 indirect-DMA gathers (128 rows each) of bf16 embedding rows with a
     zero pad row at table index 100000 -> g[tok 128p, 1152f].
  2. 9 XBAR DMA transposes (SP/Activation HWDGE) g -> embT[din, 1152] bf16.
  3. conv as 5 accumulating bf16 matmuls per 512-token tile:
     psum[dout, n] += W_k[din,dout]^T @ embT[din, n+k]
  4. DVE adds the folded bias ([dout, tok] f32) reading PSUM directly;
     gpsimd DMAs out_row[dout, 1024] f32 out. Host untransposes.
"""

import os
import sys

sys.path.insert(0, "/opt/trn_rl_repo")

import numpy as np

VOCAB = 100000
MAX_SEQ = 8192
DIM = 128
KW = 5
PAD = 2
B = 8
NCORES = 8
CHUNK = MAX_SEQ // NCORES
NBLK = 9
GATH = NBLK * 128

_CACHE = {}


def _build_nc(iters=1):
    from concourse import bacc, bass, mybir, tile

    f32 = mybir.dt.float32
    bf16 = mybir.dt.bfloat16
    i32 = mybir.dt.int32

    nc = bacc.Bacc(None, target_bir_lowering=False)
    table_d = nc.declare_dram_parameter("table", [VOCAB + 1, DIM], bf16, isOutput=False)
    idx_d = nc.declare_dram_parameter("idx", [128, B * NBLK], i32, isOutput=False)
    bias_d = nc.declare_dram_parameter("bias", [128, CHUNK], f32, isOutput=False)
    wk_d = nc.declare_dram_parameter("wk", [128, KW * DIM], bf16, isOutput=False)
    out_d = nc.declare_dram_parameter("out", [B, 128, CHUNK], f32, isOutput=True)

    with tile.TileContext(nc) as tc:
        with (
            tc.tile_pool(name="const", bufs=1) as constp,
            tc.tile_pool(name="g", bufs=3) as gpool,
            tc.tile_pool(name="embT", bufs=3) as epool,
            tc.tile_pool(name="orow", bufs=3) as orowp,
            tc.tile_pool(name="pc", bufs=4, space="PSUM") as cpool,
        ):
            idx_sb = constp.tile([128, B * NBLK], i32)
            nc.sync.dma_start(out=idx_sb[:, :], in_=idx_d[:, :])
            bias_sb = constp.tile([128, CHUNK], f32)
            nc.sync.dma_start(out=bias_sb[:, :], in_=bias_d[:, :])
            wk_sb = constp.tile([128, KW * DIM], bf16)
            nc.sync.dma_start(out=wk_sb[:, :], in_=wk_d[:, :])

            import contextlib

            loop_cm = (
                tc.For_i(0, iters, 1, hint_engines=(mybir.EngineType.PE,))
                if iters > 1
                else contextlib.nullcontext()
            )
            with loop_cm:
                body(nc, tc, bass, mybir, idx_sb, bias_sb, wk_sb,
                     table_d, out_d, gpool, epool, orowp, cpool)
    if not nc.is_finalized():
        nc.finalize()
    return nc


def body(nc, tc, bass, mybir, idx_sb, bias_sb, wk_sb, table_d, out_d,
         gpool, epool, orowp, cpool):
    f32 = mybir.dt.float32
    bf16 = mybir.dt.bfloat16
    for b in range(B):
        g = gpool.tile([128, GATH], bf16)
        for blk in range(NBLK):
            col = b * NBLK + blk
            nc.gpsimd.indirect_dma_start(
                out=g[:, blk * 128:(blk + 1) * 128],
                out_offset=None,
                in_=table_d[:, :],
                in_offset=bass.IndirectOffsetOnAxis(
                    ap=idx_sb[:, col:col + 1], axis=0
                ),
            )
        embT = epool.tile([128, GATH], bf16)
        for blk in range(NBLK):
            eng = nc.sync if blk % 2 == 0 else nc.scalar
            eng.dma_start_transpose(
                out=embT[:, blk * 128:(blk + 1) * 128],
                in_=g[:, blk * 128:(blk + 1) * 128],
            )
        out_row = orowp.tile([128, CHUNK], f32)
        for t in range(2):
            pc = cpool.tile([128, 512], f32)
            for k in range(KW):
                nc.tensor.matmul(
                    out=pc[:, :],
                    lhsT=wk_sb[:, k * DIM:(k + 1) * DIM],
                    rhs=embT[:, t * 512 + k: t * 512 + k + 512],
                    start=(k == 0),
                    stop=(k == KW - 1),
                )
            nc.vector.tensor_add(
                out_row[:, t * 512:(t + 1) * 512],
                pc[:, :],
                bias_sb[:, t * 512:(t + 1) * 512],
            )
        nc.gpsimd.dma_start(out=out_d[b], in_=out_row[:, :])
    return nc


def _prep_inputs(X, W_lin, b_lin, W_conv, b_conv, pos_table):
    import ml_dtypes

    bf16 = ml_dtypes.bfloat16

    X = np.asarray(X)
    W_lin = np.asarray(W_lin, dtype=np.float32)
    b_lin = np.asarray(b_lin, dtype=np.float32)
    W_conv = np.asarray(W_conv, dtype=np.float32)
    b_conv = np.asarray(b_conv, dtype=np.float32)
    pos_table = np.asarray(pos_table, dtype=np.float32)

    table = np.empty((VOCAB + 1, DIM), dtype=bf16)
    table[:VOCAB] = W_lin.T.astype(bf16)
    table[VOCAB] = 0.0

    wb = np.einsum("oik,i->ko", W_conv, b_lin)
    conv_lin = np.broadcast_to(wb.sum(0), (MAX_SEQ, DIM)).copy()
    conv_lin[0] = wb[2:].sum(0)
    conv_lin[1] = wb[1:].sum(0)
    conv_lin[MAX_SEQ - 2] = wb[:4].sum(0)
    conv_lin[MAX_SEQ - 1] = wb[:3].sum(0)
    bias_total = conv_lin + b_conv[None, :] + pos_table

    wk_arr = np.ascontiguousarray(
        W_conv.transpose(1, 2, 0).reshape(DIM, KW * DIM)
    ).astype(bf16)

    Xi = X.astype(np.int64)
    j = np.arange(GATH)
    in_maps = []
    for c in range(NCORES):
        a = c * CHUNK + j - PAD
        valid = (a >= 0) & (a < MAX_SEQ)
        gi = np.where(valid[None, :], Xi[:, np.clip(a, 0, MAX_SEQ - 1)], VOCAB)
        idx_c = np.ascontiguousarray(
            gi.reshape(B, NBLK, 128).transpose(2, 0, 1).reshape(128, B * NBLK)
        ).astype(np.int32)
        bias_c = np.ascontiguousarray(bias_total[c * CHUNK:(c + 1) * CHUNK].T)
        in_maps.append({"table": table, "idx": idx_c, "bias": bias_c, "wk": wk_arr})
    return in_maps


def kernel(X, W_lin, b_lin, W_conv, b_conv, pos_table):
    from concourse.bass_utils import run_bass_kernel_spmd

    iters = int(os.environ.get("KERNEL_ITERS", "1"))
    key = ("nc", iters)
    if key not in _CACHE:
        _CACHE[key] = _build_nc(iters)
    nc = _CACHE[key]

    in_maps = _prep_inputs(X, W_lin, b_lin, W_conv, b_conv, pos_table)
    res = run_bass_kernel_spmd(nc, in_maps, core_ids=list(range(NCORES)))
    _CACHE["last_results"] = res

    full = np.empty((B, MAX_SEQ, DIM), dtype=np.float32)
    for c in range(NCORES):
        o = res.results[c]["out"]
        full[:, c * CHUNK:(c + 1) * CHUNK, :] = o.transpose(0, 2, 1)
    return full



# revision 12
# speedup vs baseline: 1.8046x; 1.8046x over previous
"""Trainium2 Bass kernel: embedding gather + Conv1d(k=5,pad=2) + positional add.

Sequence-parallel across 8 NeuronCores; core c computes output tokens
[c*1024, (c+1)*1024) for all 8 batch rows. b_lin/b_conv/pos_table fold into
one per-position bias table on the host.

Gather strategy (dma_gather, mlp gpsimd library, <=1024 idxs/instr):
  The vocab table is paired: table2[w] = rows (2w, 2w+1) as one 512B entry,
  so w = v>>1. int16 idx reach forces two ranges: A = v < 65536 (idx = v>>1
  <= 32767), B = v >= 65536 (idx = (v-65536)>>1, in_ap offset 32768 pairs).

  Per group of 2 batch rows (2304 tokens):
   L1: tokens sorted A-first into 3072 slots (A cap 2048, B cap 1024); three
       1024-idx non-transpose dma_gathers (A0 A1 from table2[:32768], B0 from
       table2[32768:]) land slot s at partition s%128, rank s//128 (512B).
   L2: three SBUF-source transpose dma_gathers (1024+1024+256 idxs) produce
       embT[din, 2304] in token order; idx = (2*(slot>>7) + (v&1))*128 +
       slot%128 selects the correct 256B half of the pair (parity folded
       into the rank bits).
  Conv: 5 accumulating bf16 matmuls per 512-token PSUM tile (k-outer);
  DVE adds folded bias from PSUM; HWDGE stores out_row[dout,1024] f32.
  Host untransposes.
"""

import os
import sys

sys.path.insert(0, "/opt/trn_rl_repo")

import numpy as np

VOCAB = 100000
MAX_SEQ = 8192
DIM = 128
KW = 5
PAD = 2
B = 8
NCORES = 8
CHUNK = MAX_SEQ // NCORES
NBLK = 9
GATH = NBLK * 128          # 1152 gather slots per batch row
NGRP = 4                   # groups of 2 batch rows
GTOK = 2 * GATH            # 2304 tokens per group
CAP_A = 2048               # slots for range A per group (2 instrs)
CAP_B = 1024               # slots for range B per group (1 instr)
SLOTS = CAP_A + CAP_B      # 3072 slots -> 24 ranks of 128
NPAIR_A = 32768            # pair rows in range A
NPAIR = 50001              # total pair rows (vocab 100001 rows padded to 100002)
L1_COLS = SLOTS // 16      # idx1 cols per group (192)
L2_SPLIT = (896, 896, 512)  # transpose-mode dma_gather caps at ~896 idxs/instr
L2_COLS = GTOK // 16       # idx2 cols per group (144)

_CACHE = {}


def _build_nc(iters=1):
    from concourse import bacc, bass, mybir, tile, library_config

    f32 = mybir.dt.float32
    bf16 = mybir.dt.bfloat16
    i16 = mybir.dt.int16

    nc = bacc.Bacc(None, target_bir_lowering=False)
    table2_d = nc.declare_dram_parameter("table2", [NPAIR, 256], bf16, isOutput=False)
    idx1_d = nc.declare_dram_parameter("idx1", [128, NGRP * L1_COLS], i16, isOutput=False)
    idx2_d = nc.declare_dram_parameter("idx2", [128, NGRP * L2_COLS], i16, isOutput=False)
    bias_d = nc.declare_dram_parameter("bias", [128, CHUNK], f32, isOutput=False)
    wk_d = nc.declare_dram_parameter("wk", [128, KW * DIM], bf16, isOutput=False)
    out_d = nc.declare_dram_parameter("out", [B, 128, CHUNK], f32, isOutput=True)

    with tile.TileContext(nc) as tc:
        with (
            tc.tile_pool(name="const", bufs=1) as constp,
            tc.tile_pool(name="stag", bufs=2) as spool,
            tc.tile_pool(name="embT", bufs=2) as epool,
            tc.tile_pool(name="orow", bufs=3) as orowp,
            tc.tile_pool(name="pc", bufs=4, space="PSUM") as cpool,
        ):
            nc.gpsimd.load_library(library_config.mlp)
            idx1_sb = constp.tile([128, NGRP * L1_COLS], i16)
            nc.sync.dma_start(out=idx1_sb[:, :], in_=idx1_d[:, :])
            idx2_sb = constp.tile([128, NGRP * L2_COLS], i16)
            nc.sync.dma_start(out=idx2_sb[:, :], in_=idx2_d[:, :])
            bias_sb = constp.tile([128, CHUNK], f32)
            nc.sync.dma_start(out=bias_sb[:, :], in_=bias_d[:, :])
            wk_sb = constp.tile([128, KW * DIM], bf16)
            nc.sync.dma_start(out=wk_sb[:, :], in_=wk_d[:, :])

            import contextlib

            loop_cm = (
                tc.For_i(0, iters, 1, hint_engines=(mybir.EngineType.PE,))
                if iters > 1
                else contextlib.nullcontext()
            )
            with loop_cm:
                body(nc, tc, bass, mybir, idx1_sb, idx2_sb, bias_sb, wk_sb,
                     table2_d, out_d, spool, epool, orowp, cpool)
    if not nc.is_finalized():
        nc.finalize()
    return nc


def body(nc, tc, bass, mybir, idx1_sb, idx2_sb, bias_sb, wk_sb, table2_d,
         out_d, spool, epool, orowp, cpool):
    f32 = mybir.dt.float32
    bf16 = mybir.dt.bfloat16
    for g in range(B // 2):
        stag = spool.tile([128, SLOTS * 2], bf16)  # 24 ranks x 256 elems
        st3 = stag[:, :].rearrange("p (a b) -> p a b", a=SLOTS // 128)
        i1 = g * L1_COLS
        # L1: A0, A1 (pairs 0..32767), B0 (pairs 32768..)
        for j, (rk0, base) in enumerate([(0, 0), (8, 0), (16, NPAIR_A)]):
            nc.gpsimd.dma_gather(
                out_ap=st3[:, rk0:rk0 + 8, :],
                in_ap=table2_d[base:NPAIR_A if base == 0 else NPAIR, :],
                idxs_ap=idx1_sb[:, i1 + j * 64:i1 + (j + 1) * 64],
                num_idxs=1024,
                num_idxs_reg=1024,
                elem_size=256,
            )
        embT = epool.tile([128, GTOK], bf16)
        c0 = 0
        i2 = g * L2_COLS
        for n in L2_SPLIT:
            nc.gpsimd.dma_gather(
                out_ap=embT[:, c0:c0 + n].rearrange("p (a b) -> p a b", a=1),
                in_ap=stag[:, :],
                idxs_ap=idx2_sb[:, i2:i2 + n // 16],
                num_idxs=n,
                num_idxs_reg=n,
                elem_size=128,
                transpose=True,
                sbuf_tokens_per_rank=128,
                sbuf_free_dim_per_rank=256,
                sbuf_free_dim_pad_per_rank=0,
                sbuf_byte_offset=0,
            )
            c0 += n
            i2 += n // 16
        for bl in range(2):
            b = 2 * g + bl
            out_row = orowp.tile([128, CHUNK], f32)
            pcs = [cpool.tile([128, 512], f32, name=f"pc{t}") for t in range(2)]
            for k in range(KW):
                for t in range(2):
                    nc.tensor.matmul(
                        out=pcs[t][:, :],
                        lhsT=wk_sb[:, k * DIM:(k + 1) * DIM],
                        rhs=embT[:, bl * GATH + t * 512 + k: bl * GATH + t * 512 + k + 512],
                        start=(k == 0),
                        stop=(k == KW - 1),
                    )
            for t in range(2):
                nc.vector.tensor_add(
                    out_row[:, t * 512:(t + 1) * 512],
                    pcs[t][:, :],
                    bias_sb[:, t * 512:(t + 1) * 512],
                )
            oeng = nc.scalar if b % 2 == 0 else nc.sync
            oeng.dma_start(out=out_d[b], in_=out_row[:, :])
    return nc


def _wrap16(v):
    """[n] int16 -> [128, n/16]: index i at [i%16, i//16], replicated to all
    8 groups of 16 partitions (each Q7 core reads its own 16)."""
    a = v.reshape(-1, 16).T
    return np.tile(a, (8, 1)).astype(np.int16)


def _prep_inputs(X, W_lin, b_lin, W_conv, b_conv, pos_table):
    import ml_dtypes

    bf16 = ml_dtypes.bfloat16

    X = np.asarray(X)
    W_lin = np.asarray(W_lin, dtype=np.float32)
    b_lin = np.asarray(b_lin, dtype=np.float32)
    W_conv = np.asarray(W_conv, dtype=np.float32)
    b_conv = np.asarray(b_conv, dtype=np.float32)
    pos_table = np.asarray(pos_table, dtype=np.float32)

    flat = np.zeros((2 * NPAIR, DIM), dtype=bf16)
    flat[:VOCAB] = W_lin.T.astype(bf16)          # rows VOCAB..100001 stay zero
    table2 = np.ascontiguousarray(flat.reshape(NPAIR, 256))

    wb = np.einsum("oik,i->ko", W_conv, b_lin)
    conv_lin = np.broadcast_to(wb.sum(0), (MAX_SEQ, DIM)).copy()
    conv_lin[0] = wb[2:].sum(0)
    conv_lin[1] = wb[1:].sum(0)
    conv_lin[MAX_SEQ - 2] = wb[:4].sum(0)
    conv_lin[MAX_SEQ - 1] = wb[:3].sum(0)
    bias_total = conv_lin + b_conv[None, :] + pos_table

    wk_arr = np.ascontiguousarray(
        W_conv.transpose(1, 2, 0).reshape(DIM, KW * DIM)
    ).astype(bf16)

    Xi = X.astype(np.int64)
    j = np.arange(GATH)
    in_maps = []
    for c in range(NCORES):
        a = c * CHUNK + j - PAD
        valid = (a >= 0) & (a < MAX_SEQ)
        gi = np.where(valid[None, :], Xi[:, np.clip(a, 0, MAX_SEQ - 1)], VOCAB)
        # gi[b, i]: vocab row for gather slot i of batch b (pad -> VOCAB=zeros)
        idx1_cols = []
        idx2_cols = []
        for g in range(NGRP):
            toks = gi[2 * g:2 * g + 2].reshape(GTOK)   # token order (b_local, i)
            isA = toks < 2 * NPAIR_A
            nA = int(isA.sum())
            nB = GTOK - nA
            if nA > CAP_A or nB > CAP_B:
                raise ValueError(f"range overflow: nA={nA} nB={nB}")
            l1 = np.zeros(SLOTS, dtype=np.int16)
            l1[:nA] = (toks[isA] >> 1).astype(np.int16)
            l1[CAP_A:CAP_A + nB] = ((toks[~isA] - 2 * NPAIR_A) >> 1).astype(np.int16)
            slot = np.empty(GTOK, dtype=np.int64)
            slot[isA] = np.arange(nA)
            slot[~isA] = CAP_A + np.arange(nB)
            l2 = ((2 * (slot >> 7) + (toks & 1)) * 128 + (slot & 127)).astype(np.int16)
            for seg in range(3):
                idx1_cols.append(_wrap16(l1[seg * 1024:(seg + 1) * 1024]))
            c0 = 0
            for n in L2_SPLIT:
                idx2_cols.append(_wrap16(l2[c0:c0 + n]))
                c0 += n
        idx1_c = np.ascontiguousarray(np.concatenate(idx1_cols, axis=1))
        idx2_c = np.ascontiguousarray(np.concatenate(idx2_cols, axis=1))
        bias_c = np.ascontiguousarray(bias_total[c * CHUNK:(c + 1) * CHUNK].T)
        in_maps.append({"table2": table2, "idx1": idx1_c, "idx2": idx2_c,
                        "bias": bias_c, "wk": wk_arr})
    return in_maps


def kernel(X, W_lin, b_lin, W_conv, b_conv, pos_table):
    from concourse.bass_utils import run_bass_kernel_spmd

    iters = int(os.environ.get("KERNEL_ITERS", "1"))
    key = ("nc", iters)
    if key not in _CACHE:
        _CACHE[key] = _build_nc(iters)
    nc = _CACHE[key]

    in_maps = _prep_inputs(X, W_lin, b_lin, W_conv, b_conv, pos_table)
    res = run_bass_kernel_spmd(nc, in_maps, core_ids=list(range(NCORES)))
    _CACHE["last_results"] = res

    full = np.empty((B, MAX_SEQ, DIM), dtype=np.float32)
    for c in range(NCORES):
        o = res.results[c]["out"]
        full[:, c * CHUNK:(c + 1) * CHUNK, :] = o.transpose(0, 2, 1)
    return full


# revision 14
# speedup vs baseline: 2.3799x; 1.3188x over previous
"""Trainium2 Bass kernel: embedding gather + Conv1d(k=5,pad=2) + positional add.

Sequence-parallel across 8 NeuronCores; core c computes output tokens
[c*1024, (c+1)*1024) for all 8 batch rows. b_lin/b_conv/pos_table fold into
one per-position bias table on the host.

Per core, per batch row:
  1. ONE indirect-DMA gather (1152 rows, offset ap [128, 9]) of bf16
     embedding rows with a zero pad row at table index 100000
     -> g[tok 128p, 1152f]  (g[p, j*128+d] = row of token j*128+p).
  2. ONE XBAR DMA transpose (HWDGE) g -> embT[din, 9, 128] (blockwise:
     embT[d, j, t] = g[t, j*128+d], i.e. [din, token] flat).
  3. conv as 5 accumulating bf16 matmuls per 512-token tile (k-outer over
     both tiles to share LDWEIGHTS):
     psum[dout, n] += W_k[din,dout]^T @ embT[din, n+k]
  4. DVE adds the folded bias ([dout, tok] f32) reading PSUM directly;
     HWDGE DMAs out_row[dout, 1024] f32 out. Host untransposes.
"""

import os
import sys

sys.path.insert(0, "/opt/trn_rl_repo")

import numpy as np

VOCAB = 100000
MAX_SEQ = 8192
DIM = 128
KW = 5
PAD = 2
B = 8
NCORES = 8
CHUNK = MAX_SEQ // NCORES
NBLK = 9
GATH = NBLK * 128

_CACHE = {}


def _build_nc(iters=1):
    from concourse import bacc, bass, mybir, tile

    f32 = mybir.dt.float32
    bf16 = mybir.dt.bfloat16
    i32 = mybir.dt.int32

    nc = bacc.Bacc(None, target_bir_lowering=False)
    table_d = nc.declare_dram_parameter("table", [VOCAB + 1, DIM], bf16, isOutput=False)
    idx_d = nc.declare_dram_parameter("idx", [128, B * NBLK], i32, isOutput=False)
    bias_d = nc.declare_dram_parameter("bias", [128, CHUNK], f32, isOutput=False)
    wk_d = nc.declare_dram_parameter("wk", [128, KW * DIM], bf16, isOutput=False)
    out_d = nc.declare_dram_parameter("out", [B, 128, CHUNK], f32, isOutput=True)

    with tile.TileContext(nc) as tc:
        with (
            tc.tile_pool(name="const", bufs=1) as constp,
            tc.tile_pool(name="g", bufs=8) as gpool,
            tc.tile_pool(name="embT", bufs=4) as epool,
            tc.tile_pool(name="orow", bufs=4) as orowp,
            tc.tile_pool(name="pc", bufs=4, space="PSUM") as cpool,
        ):
            idx_sb = constp.tile([128, B * NBLK], i32)
            nc.sync.dma_start(out=idx_sb[:, :], in_=idx_d[:, :])
            bias_sb = constp.tile([128, CHUNK], f32)
            nc.sync.dma_start(out=bias_sb[:, :], in_=bias_d[:, :])
            wk_sb = constp.tile([128, KW * DIM], bf16)
            nc.sync.dma_start(out=wk_sb[:, :], in_=wk_d[:, :])

            import contextlib

            loop_cm = (
                tc.For_i(0, iters, 1, hint_engines=(mybir.EngineType.PE,))
                if iters > 1
                else contextlib.nullcontext()
            )
            with loop_cm:
                body(nc, tc, bass, mybir, idx_sb, bias_sb, wk_sb,
                     table_d, out_d, gpool, epool, orowp, cpool)
    if not nc.is_finalized():
        nc.finalize()
    return nc


def body(nc, tc, bass, mybir, idx_sb, bias_sb, wk_sb, table_d, out_d,
         gpool, epool, orowp, cpool):
    f32 = mybir.dt.float32
    bf16 = mybir.dt.bfloat16
    for b in range(B):
        g = gpool.tile([128, GATH], bf16)
        # The SWDGE ucode consumes ONE offset per partition (per contiguous
        # dest run), so each instruction gathers 128 rows into one 128-col
        # block. 9 instructions per batch row.
        for blk in range(NBLK):
            col = b * NBLK + blk
            nc.gpsimd.indirect_dma_start(
                out=g[:, blk * 128:(blk + 1) * 128],
                out_offset=None,
                in_=table_d[:, :],
                in_offset=bass.IndirectOffsetOnAxis(
                    ap=idx_sb[:, col:col + 1], axis=0
                ),
            )
        embT = epool.tile([128, GATH], bf16)
        teng = nc.sync if b % 2 == 0 else nc.scalar
        teng.dma_start_transpose(
            out=embT[:, :].rearrange("p (a b) -> p a b", a=NBLK),
            in_=g[:, :],
        )
        out_row = orowp.tile([128, CHUNK], f32)
        pcs = [cpool.tile([128, 512], f32, name=f"pc{t}") for t in range(2)]
        for k in range(KW):
            for t in range(2):
                nc.tensor.matmul(
                    out=pcs[t][:, :],
                    lhsT=wk_sb[:, k * DIM:(k + 1) * DIM],
                    rhs=embT[:, t * 512 + k: t * 512 + k + 512],
                    start=(k == 0),
                    stop=(k == KW - 1),
                )
        for t in range(2):
            nc.vector.tensor_add(
                out_row[:, t * 512:(t + 1) * 512],
                pcs[t][:, :],
                bias_sb[:, t * 512:(t + 1) * 512],
            )
        oeng = nc.scalar if b % 2 == 0 else nc.sync
        oeng.dma_start(out=out_d[b], in_=out_row[:, :])
    return nc


def _prep_inputs(X, W_lin, b_lin, W_conv, b_conv, pos_table):
    import ml_dtypes

    bf16 = ml_dtypes.bfloat16

    X = np.asarray(X)
    W_lin = np.asarray(W_lin, dtype=np.float32)
    b_lin = np.asarray(b_lin, dtype=np.float32)
    W_conv = np.asarray(W_conv, dtype=np.float32)
    b_conv = np.asarray(b_conv, dtype=np.float32)
    pos_table = np.asarray(pos_table, dtype=np.float32)

    table = np.empty((VOCAB + 1, DIM), dtype=bf16)
    table[:VOCAB] = W_lin.T.astype(bf16)
    table[VOCAB] = 0.0

    wb = np.einsum("oik,i->ko", W_conv, b_lin)
    conv_lin = np.broadcast_to(wb.sum(0), (MAX_SEQ, DIM)).copy()
    conv_lin[0] = wb[2:].sum(0)
    conv_lin[1] = wb[1:].sum(0)
    conv_lin[MAX_SEQ - 2] = wb[:4].sum(0)
    conv_lin[MAX_SEQ - 1] = wb[:3].sum(0)
    bias_total = conv_lin + b_conv[None, :] + pos_table

    wk_arr = np.ascontiguousarray(
        W_conv.transpose(1, 2, 0).reshape(DIM, KW * DIM)
    ).astype(bf16)

    Xi = X.astype(np.int64)
    j = np.arange(GATH)
    in_maps = []
    for c in range(NCORES):
        a = c * CHUNK + j - PAD
        valid = (a >= 0) & (a < MAX_SEQ)
        gi = np.where(valid[None, :], Xi[:, np.clip(a, 0, MAX_SEQ - 1)], VOCAB)
        idx_c = np.ascontiguousarray(
            gi.reshape(B, NBLK, 128).transpose(2, 0, 1).reshape(128, B * NBLK)
        ).astype(np.int32)
        bias_c = np.ascontiguousarray(bias_total[c * CHUNK:(c + 1) * CHUNK].T)
        in_maps.append({"table": table, "idx": idx_c, "bias": bias_c, "wk": wk_arr})
    return in_maps


def kernel(X, W_lin, b_lin, W_conv, b_conv, pos_table):
    from concourse.bass_utils import run_bass_kernel_spmd

    iters = int(os.environ.get("KERNEL_ITERS", "1"))
    key = ("nc", iters)
    if key not in _CACHE:
        _CACHE[key] = _build_nc(iters)
    nc = _CACHE[key]

    in_maps = _prep_inputs(X, W_lin, b_lin, W_conv, b_conv, pos_table)
    res = run_bass_kernel_spmd(nc, in_maps, core_ids=list(range(NCORES)))
    _CACHE["last_results"] = res

    full = np.empty((B, MAX_SEQ, DIM), dtype=np.float32)
    for c in range(NCORES):
        o = res.results[c]["out"]
        full[:, c * CHUNK:(c + 1) * CHUNK, :] = o.transpose(0, 2, 1)
    return full


# revision 16
# speedup vs baseline: 2.6250x; 1.1030x over previous
"""Trainium2 Bass kernel: embedding gather + Conv1d(k=5,pad=2) + positional add.

Sequence-parallel across 8 NeuronCores; core c computes output tokens
[c*1024, (c+1)*1024) for all 8 batch rows. b_lin/b_conv/pos_table fold into
one per-position bias table on the host.

Per core, per batch row:
  1. ONE indirect-DMA gather (1152 rows, offset ap [128, 9]) of bf16
     embedding rows with a zero pad row at table index 100000
     -> g[tok 128p, 1152f]  (g[p, j*128+d] = row of token j*128+p).
  2. ONE XBAR DMA transpose (HWDGE) g -> embT[din, 9, 128] (blockwise:
     embT[d, j, t] = g[t, j*128+d], i.e. [din, token] flat).
  3. conv as 5 accumulating bf16 matmuls per 512-token tile (k-outer over
     both tiles to share LDWEIGHTS):
     psum[dout, n] += W_k[din,dout]^T @ embT[din, n+k]
  4. DVE adds the folded bias ([dout, tok] f32) reading PSUM directly;
     HWDGE DMAs out_row[dout, 1024] f32 out. Host untransposes.
"""

import os
import sys

sys.path.insert(0, "/opt/trn_rl_repo")

import numpy as np

VOCAB = 100000
MAX_SEQ = 8192
DIM = 128
KW = 5
PAD = 2
B = 8
NCORES = 8
CHUNK = MAX_SEQ // NCORES
NBLK = 9
GATH = NBLK * 128

_CACHE = {}


def _build_nc(iters=1):
    from concourse import bacc, bass, mybir, tile

    f32 = mybir.dt.float32
    bf16 = mybir.dt.bfloat16
    i32 = mybir.dt.int32

    nc = bacc.Bacc(None, target_bir_lowering=False)
    table_d = nc.declare_dram_parameter("table", [VOCAB + 1, DIM], bf16, isOutput=False)
    idx_d = nc.declare_dram_parameter("idx", [128, B * NBLK], i32, isOutput=False)
    bias_d = nc.declare_dram_parameter("bias", [128, CHUNK], f32, isOutput=False)
    wk_d = nc.declare_dram_parameter("wk", [128, KW * DIM], bf16, isOutput=False)
    out_d = nc.declare_dram_parameter("out", [B, 128, CHUNK], f32, isOutput=True)

    with tile.TileContext(nc) as tc:
        with (
            tc.tile_pool(name="const", bufs=1) as constp,
            tc.tile_pool(name="g", bufs=4) as gpool,
            tc.tile_pool(name="embT", bufs=3) as epool,
            tc.tile_pool(name="orow", bufs=4) as orowp,
            tc.tile_pool(name="pc", bufs=4, space="PSUM") as cpool,
        ):
            idx_sb = constp.tile([128, B * NBLK], i32)
            nc.sync.dma_start(out=idx_sb[:, :], in_=idx_d[:, :])
            bias_sb = constp.tile([128, CHUNK], f32)
            nc.sync.dma_start(out=bias_sb[:, :], in_=bias_d[:, :])
            wk_sb = constp.tile([128, KW * DIM], bf16)
            nc.sync.dma_start(out=wk_sb[:, :], in_=wk_d[:, :])

            import contextlib

            loop_cm = (
                tc.For_i(0, iters, 1, hint_engines=(mybir.EngineType.PE,))
                if iters > 1
                else contextlib.nullcontext()
            )
            with loop_cm:
                body(nc, tc, bass, mybir, idx_sb, bias_sb, wk_sb,
                     table_d, out_d, gpool, epool, orowp, cpool)
    if not nc.is_finalized():
        nc.finalize()
    return nc


def body(nc, tc, bass, mybir, idx_sb, bias_sb, wk_sb, table_d, out_d,
         gpool, epool, orowp, cpool):
    f32 = mybir.dt.float32
    bf16 = mybir.dt.bfloat16
    GB = 2  # batch rows per transpose group (fewer XBAR events = fewer
    #         SWDGE-vs-transpose serialization stalls)
    for grp in range(B // GB):
        g = gpool.tile([128, GB * GATH], bf16)
        # The SWDGE ucode consumes ONE offset per partition (per contiguous
        # dest run), so each instruction gathers 128 rows into one 128-col
        # block. 9 instructions per batch row.
        for bl in range(GB):
            b = grp * GB + bl
            for blk in range(NBLK):
                col = b * NBLK + blk
                nc.gpsimd.indirect_dma_start(
                    out=g[:, (bl * NBLK + blk) * 128:(bl * NBLK + blk + 1) * 128],
                    out_offset=None,
                    in_=table_d[:, :],
                    in_offset=bass.IndirectOffsetOnAxis(
                        ap=idx_sb[:, col:col + 1], axis=0
                    ),
                )
        embT = epool.tile([128, GB * GATH], bf16)
        teng = nc.sync if grp % 2 == 0 else nc.scalar
        teng.dma_start_transpose(
            out=embT[:, :].rearrange("p (a b) -> p a b", a=GB * NBLK),
            in_=g[:, :],
        )
        for bl in range(GB):
            b = grp * GB + bl
            out_row = orowp.tile([128, CHUNK], f32)
            pcs = [cpool.tile([128, 512], f32, name=f"pc{t}") for t in range(2)]
            for k in range(KW):
                for t in range(2):
                    nc.tensor.matmul(
                        out=pcs[t][:, :],
                        lhsT=wk_sb[:, k * DIM:(k + 1) * DIM],
                        rhs=embT[:, bl * GATH + t * 512 + k: bl * GATH + t * 512 + k + 512],
                        start=(k == 0),
                        stop=(k == KW - 1),
                    )
            for t in range(2):
                nc.vector.tensor_add(
                    out_row[:, t * 512:(t + 1) * 512],
                    pcs[t][:, :],
                    bias_sb[:, t * 512:(t + 1) * 512],
                )
            oeng = nc.scalar if b % 2 == 0 else nc.sync
            oeng.dma_start(out=out_d[b], in_=out_row[:, :])
    return nc


def _prep_inputs(X, W_lin, b_lin, W_conv, b_conv, pos_table):
    import ml_dtypes

    bf16 = ml_dtypes.bfloat16

    X = np.asarray(X)
    W_lin = np.asarray(W_lin, dtype=np.float32)
    b_lin = np.asarray(b_lin, dtype=np.float32)
    W_conv = np.asarray(W_conv, dtype=np.float32)
    b_conv = np.asarray(b_conv, dtype=np.float32)
    pos_table = np.asarray(pos_table, dtype=np.float32)

    table = np.empty((VOCAB + 1, DIM), dtype=bf16)
    table[:VOCAB] = W_lin.T.astype(bf16)
    table[VOCAB] = 0.0

    wb = np.einsum("oik,i->ko", W_conv, b_lin)
    conv_lin = np.broadcast_to(wb.sum(0), (MAX_SEQ, DIM)).copy()
    conv_lin[0] = wb[2:].sum(0)
    conv_lin[1] = wb[1:].sum(0)
    conv_lin[MAX_SEQ - 2] = wb[:4].sum(0)
    conv_lin[MAX_SEQ - 1] = wb[:3].sum(0)
    bias_total = conv_lin + b_conv[None, :] + pos_table

    wk_arr = np.ascontiguousarray(
        W_conv.transpose(1, 2, 0).reshape(DIM, KW * DIM)
    ).astype(bf16)

    Xi = X.astype(np.int64)
    j = np.arange(GATH)
    in_maps = []
    for c in range(NCORES):
        a = c * CHUNK + j - PAD
        valid = (a >= 0) & (a < MAX_SEQ)
        gi = np.where(valid[None, :], Xi[:, np.clip(a, 0, MAX_SEQ - 1)], VOCAB)
        idx_c = np.ascontiguousarray(
            gi.reshape(B, NBLK, 128).transpose(2, 0, 1).reshape(128, B * NBLK)
        ).astype(np.int32)
        bias_c = np.ascontiguousarray(bias_total[c * CHUNK:(c + 1) * CHUNK].T)
        in_maps.append({"table": table, "idx": idx_c, "bias": bias_c, "wk": wk_arr})
    return in_maps


def kernel(X, W_lin, b_lin, W_conv, b_conv, pos_table):
    from concourse.bass_utils import run_bass_kernel_spmd

    iters = int(os.environ.get("KERNEL_ITERS", "1"))
    key = ("nc", iters)
    if key not in _CACHE:
        _CACHE[key] = _build_nc(iters)
    nc = _CACHE[key]

    in_maps = _prep_inputs(X, W_lin, b_lin, W_conv, b_conv, pos_table)
    res = run_bass_kernel_spmd(nc, in_maps, core_ids=list(range(NCORES)))
    _CACHE["last_results"] = res

    full = np.empty((B, MAX_SEQ, DIM), dtype=np.float32)
    for c in range(NCORES):
        o = res.results[c]["out"]
        full[:, c * CHUNK:(c + 1) * CHUNK, :] = o.transpose(0, 2, 1)
    return full


# revision 17
# speedup vs baseline: 2.7225x; 1.0372x over previous
"""Trainium2 Bass kernel: embedding gather + Conv1d(k=5,pad=2) + positional add.

Sequence-parallel across 8 NeuronCores; core c computes output tokens
[c*1024, (c+1)*1024) for all 8 batch rows. b_lin/b_conv/pos_table fold into
one per-position bias table on the host.

Per core, per batch row:
  1. ONE indirect-DMA gather (1152 rows, offset ap [128, 9]) of bf16
     embedding rows with a zero pad row at table index 100000
     -> g[tok 128p, 1152f]  (g[p, j*128+d] = row of token j*128+p).
  2. ONE XBAR DMA transpose (HWDGE) g -> embT[din, 9, 128] (blockwise:
     embT[d, j, t] = g[t, j*128+d], i.e. [din, token] flat).
  3. conv as 5 accumulating bf16 matmuls per 512-token tile (k-outer over
     both tiles to share LDWEIGHTS):
     psum[dout, n] += W_k[din,dout]^T @ embT[din, n+k]
  4. DVE adds the folded bias ([dout, tok] f32) reading PSUM directly;
     HWDGE DMAs out_row[dout, 1024] f32 out. Host untransposes.
"""

import os
import sys

sys.path.insert(0, "/opt/trn_rl_repo")

import numpy as np

VOCAB = 100000
MAX_SEQ = 8192
DIM = 128
KW = 5
PAD = 2
B = 8
NCORES = 8
CHUNK = MAX_SEQ // NCORES
NBLK = 9
GATH = NBLK * 128

_CACHE = {}


def _build_nc(iters=1):
    from concourse import bacc, bass, mybir, tile

    f32 = mybir.dt.float32
    bf16 = mybir.dt.bfloat16
    i32 = mybir.dt.int32

    nc = bacc.Bacc(None, target_bir_lowering=False)
    table_d = nc.declare_dram_parameter("table", [VOCAB + 1, DIM], bf16, isOutput=False)
    idx_d = nc.declare_dram_parameter("idx", [128, B * NBLK], i32, isOutput=False)
    bias_d = nc.declare_dram_parameter("bias", [128, CHUNK], f32, isOutput=False)
    wk_d = nc.declare_dram_parameter("wk", [128, KW * DIM], bf16, isOutput=False)
    out_d = nc.declare_dram_parameter("out", [B, 128, CHUNK], f32, isOutput=True)

    with tile.TileContext(nc) as tc:
        with (
            tc.tile_pool(name="const", bufs=1) as constp,
            tc.tile_pool(name="g", bufs=4) as gpool,
            tc.tile_pool(name="embT", bufs=3) as epool,
            tc.tile_pool(name="orow", bufs=4) as orowp,
            tc.tile_pool(name="pc", bufs=4, space="PSUM") as cpool,
        ):
            idx_sb = constp.tile([128, B * NBLK], i32)
            nc.sync.dma_start(out=idx_sb[:, :], in_=idx_d[:, :])
            bias_sb = constp.tile([128, CHUNK], f32)
            nc.sync.dma_start(out=bias_sb[:, :], in_=bias_d[:, :])
            wk_sb = constp.tile([128, KW * DIM], bf16)
            nc.sync.dma_start(out=wk_sb[:, :], in_=wk_d[:, :])

            import contextlib

            loop_cm = (
                tc.For_i(0, iters, 1, hint_engines=(mybir.EngineType.PE,))
                if iters > 1
                else contextlib.nullcontext()
            )
            with loop_cm:
                body(nc, tc, bass, mybir, idx_sb, bias_sb, wk_sb,
                     table_d, out_d, gpool, epool, orowp, cpool)
    if not nc.is_finalized():
        nc.finalize()
    return nc


def body(nc, tc, bass, mybir, idx_sb, bias_sb, wk_sb, table_d, out_d,
         gpool, epool, orowp, cpool):
    f32 = mybir.dt.float32
    bf16 = mybir.dt.bfloat16
    GB = 4  # batch rows per transpose group (fewer XBAR events = fewer
    #         SWDGE-vs-transpose serialization stalls)
    for grp in range(B // GB):
        g = gpool.tile([128, GB * GATH], bf16)
        # The SWDGE ucode consumes ONE offset per partition (per contiguous
        # dest run), so each instruction gathers 128 rows into one 128-col
        # block. 9 instructions per batch row.
        for bl in range(GB):
            b = grp * GB + bl
            for blk in range(NBLK):
                col = b * NBLK + blk
                nc.gpsimd.indirect_dma_start(
                    out=g[:, (bl * NBLK + blk) * 128:(bl * NBLK + blk + 1) * 128],
                    out_offset=None,
                    in_=table_d[:, :],
                    in_offset=bass.IndirectOffsetOnAxis(
                        ap=idx_sb[:, col:col + 1], axis=0
                    ),
                )
        embT = epool.tile([128, GB * GATH], bf16)
        teng = nc.sync if grp % 2 == 0 else nc.scalar
        teng.dma_start_transpose(
            out=embT[:, :].rearrange("p (a b) -> p a b", a=GB * NBLK),
            in_=g[:, :],
        )
        for bl in range(GB):
            b = grp * GB + bl
            out_row = orowp.tile([128, CHUNK], f32)
            pcs = [cpool.tile([128, 512], f32, name=f"pc{t}") for t in range(2)]
            for k in range(KW):
                for t in range(2):
                    nc.tensor.matmul(
                        out=pcs[t][:, :],
                        lhsT=wk_sb[:, k * DIM:(k + 1) * DIM],
                        rhs=embT[:, bl * GATH + t * 512 + k: bl * GATH + t * 512 + k + 512],
                        start=(k == 0),
                        stop=(k == KW - 1),
                    )
            for t in range(2):
                nc.vector.tensor_add(
                    out_row[:, t * 512:(t + 1) * 512],
                    pcs[t][:, :],
                    bias_sb[:, t * 512:(t + 1) * 512],
                )
            oeng = nc.scalar if b % 2 == 0 else nc.sync
            oeng.dma_start(out=out_d[b], in_=out_row[:, :])
    return nc


def _prep_inputs(X, W_lin, b_lin, W_conv, b_conv, pos_table):
    import ml_dtypes

    bf16 = ml_dtypes.bfloat16

    X = np.asarray(X)
    W_lin = np.asarray(W_lin, dtype=np.float32)
    b_lin = np.asarray(b_lin, dtype=np.float32)
    W_conv = np.asarray(W_conv, dtype=np.float32)
    b_conv = np.asarray(b_conv, dtype=np.float32)
    pos_table = np.asarray(pos_table, dtype=np.float32)

    table = np.empty((VOCAB + 1, DIM), dtype=bf16)
    table[:VOCAB] = W_lin.T.astype(bf16)
    table[VOCAB] = 0.0

    wb = np.einsum("oik,i->ko", W_conv, b_lin)
    conv_lin = np.broadcast_to(wb.sum(0), (MAX_SEQ, DIM)).copy()
    conv_lin[0] = wb[2:].sum(0)
    conv_lin[1] = wb[1:].sum(0)
    conv_lin[MAX_SEQ - 2] = wb[:4].sum(0)
    conv_lin[MAX_SEQ - 1] = wb[:3].sum(0)
    bias_total = conv_lin + b_conv[None, :] + pos_table

    wk_arr = np.ascontiguousarray(
        W_conv.transpose(1, 2, 0).reshape(DIM, KW * DIM)
    ).astype(bf16)

    Xi = X.astype(np.int64)
    j = np.arange(GATH)
    in_maps = []
    for c in range(NCORES):
        a = c * CHUNK + j - PAD
        valid = (a >= 0) & (a < MAX_SEQ)
        gi = np.where(valid[None, :], Xi[:, np.clip(a, 0, MAX_SEQ - 1)], VOCAB)
        idx_c = np.ascontiguousarray(
            gi.reshape(B, NBLK, 128).transpose(2, 0, 1).reshape(128, B * NBLK)
        ).astype(np.int32)
        bias_c = np.ascontiguousarray(bias_total[c * CHUNK:(c + 1) * CHUNK].T)
        in_maps.append({"table": table, "idx": idx_c, "bias": bias_c, "wk": wk_arr})
    return in_maps


def kernel(X, W_lin, b_lin, W_conv, b_conv, pos_table):
    from concourse.bass_utils import run_bass_kernel_spmd

    iters = int(os.environ.get("KERNEL_ITERS", "1"))
    key = ("nc", iters)
    if key not in _CACHE:
        _CACHE[key] = _build_nc(iters)
    nc = _CACHE[key]

    in_maps = _prep_inputs(X, W_lin, b_lin, W_conv, b_conv, pos_table)
    res = run_bass_kernel_spmd(nc, in_maps, core_ids=list(range(NCORES)))
    _CACHE["last_results"] = res

    full = np.empty((B, MAX_SEQ, DIM), dtype=np.float32)
    for c in range(NCORES):
        o = res.results[c]["out"]
        full[:, c * CHUNK:(c + 1) * CHUNK, :] = o.transpose(0, 2, 1)
    return full
